# revision 1
# baseline (speedup 1.0000x reference)
"""Trainium2 Bass kernel for nn_CrossAttention_61890478735686.

Math per (batch n, unit u):
    q = query[n] viewed [c=256, hw=256];  raw DRAM layout [hw, c] = q^T
    k = v = value[n] same.
    qW = q @ Wq[u]   [256, 64]
    kW = k @ Wk[u]   [256, 64]
    dot = qW @ kW^T  [256, 256];  attn = softmax(dot/16, axis=-1)
    vW = k @ Wv[u]   [256, 9]
    out = attn @ vW  [256, 9] -> output[n, kh, kw, c, u], m = 3*kh+kw

Kernel dataflow (everything transposed so softmax reduction is the
contraction axis of the final matmul):
    qWT[q, c]  = Wq[u]^T @ q^T      (lhsT = Wq chunks, rhs = raw query)
    kWT[q, c]  = Wk[u]^T @ k^T
    dotT[d, c] = kWT^T-contraction over q (lhsT = kWT cols, rhs = qWT)
    ET = exp(dotT / 16)             (ACT, PSUM -> SBUF)
    unnorm[m, c] = vW_aug^T @ ET    (vW augmented with a ones column ->
                                     row 9 = softmax denominator S[c])
    host: out = unnorm[:9] / unnorm[9]

Sharding: tensor-parallel over units. Core i gets units 16i..16i+16 and
all 16 batches (256 (n,u) pairs per core).
"""

import sys

if "/opt/trn_rl_repo" not in sys.path:
    sys.path.insert(0, "/opt/trn_rl_repo")

import numpy as np

import concourse.bass as bass
import concourse.tile as tile
from concourse import mybir
from concourse.bass_utils import run_bass_kernel_spmd

F32 = mybir.dt.float32
F32R = mybir.dt.float32r

N_CORES = 8
NB = 16          # batches
UPC = 16         # units per core
C = 256          # channels
HW = 256         # h*w (contraction dim of the projections)
QK = 64          # qk_dim
M = 9            # kernel_len
MA = 10          # M + ones column
SCALE = 1.0 / 16.0

USE_F32R = True
MMDT = F32R if USE_F32R else F32
ETDT = MMDT


def split_multiwait_drains(nc):
    """This walrus build cannot codegen instructions carrying >1 sem wait
    (CoreV3GenImpl setupSyncWait: 'Too many sync wait commands').  Hoist
    all but the last wait into single-wait NOPs preceding the instruction
    on the same engine — semantically identical (the sequencer stalls on
    each in turn)."""
    for f in nc.m.functions:
        for bb in f.blocks:
            new_insts = []
            for inst in bb.instructions:
                si = getattr(inst, "sync_info", None)
                if si is not None and len(si.on_wait) > 1:
                    waits = list(si.on_wait)
                    for j, w in enumerate(waits[:-1]):
                        nop = mybir.InstNoOp(
                            name=f"{inst.name}-wsplit{j}",
                            engine=inst.engine,
                            ins=[],
                            outs=[],
                            sync_info=mybir.SyncInfo(on_wait=[w], on_update=[]),
                        )
                        new_insts.append(nop)
                    si.on_wait = [waits[-1]]
                new_insts.append(inst)
            bb.instructions = new_insts


def build_nc():
    nc = bass.Bass()

    q_d = nc.dram_tensor("query", [NB, HW, C], MMDT, kind="ExternalInput")
    v_d = nc.dram_tensor("value", [NB, HW, C], MMDT, kind="ExternalInput")
    wq_d = nc.dram_tensor("query_w", [UPC, HW, QK], MMDT, kind="ExternalInput")
    wk_d = nc.dram_tensor("key_w", [UPC, HW, QK], MMDT, kind="ExternalInput")
    wv_d = nc.dram_tensor("value_w", [UPC, HW, M], MMDT, kind="ExternalInput")
    ones_d = nc.dram_tensor("ones", [128, 1], MMDT, kind="ExternalInput")
    out_d = nc.dram_tensor("out", [NB, UPC, MA, C], F32, kind="ExternalOutput")

    with tile.TileContext(nc) as tc:
        with (
            tc.tile_pool(name="persist", bufs=1) as persist,
            tc.tile_pool(name="kqp", bufs=3) as kqp,
            tc.tile_pool(name="etp", bufs=4) as etp,
            tc.tile_pool(name="augp", bufs=3) as augp,
            tc.tile_pool(name="outp", bufs=3) as outp,
            tc.tile_pool(name="ps_qk", bufs=1, space="PSUM") as ps_qk,
            tc.tile_pool(name="ps_dot", bufs=2, space="PSUM") as ps_dot,
            tc.tile_pool(name="ps_vw", bufs=1, space="PSUM") as ps_vw,
            tc.tile_pool(name="ps_out", bufs=1, space="PSUM") as ps_out,
        ):
            # ---- persistent inputs --------------------------------------
            # q_sb/v_sb: [p=128, n, k, c]; rows (k*128+p) of raw [hw, c]
            q_sb = persist.tile([128, NB, 2, C], MMDT)
            v_sb = persist.tile([128, NB, 2, C], MMDT)
            # wq/wk: [p, pair, k, (u2 qk)]  -> lhsT [128, 128] slices
            wq_sb = persist.tile([128, UPC // 2, 2, 2 * QK], MMDT)
            wk_sb = persist.tile([128, UPC // 2, 2, 2 * QK], MMDT)
            # wv: [p, k, u, m]
            wv_sb = persist.tile([128, 2, UPC, M], MMDT)
            ones_sb = persist.tile([128, 1], MMDT)
            nc.sync.dma_start(out=ones_sb[:], in_=ones_d[:])

            for n in range(NB):
                nc.sync.dma_start(
                    out=q_sb[:, n], in_=q_d[n].rearrange("(k p) c -> p k c", p=128)
                )
                nc.sync.dma_start(
                    out=v_sb[:, n], in_=v_d[n].rearrange("(k p) c -> p k c", p=128)
                )
            for pr in range(UPC // 2):
                for k in range(2):
                    nc.sync.dma_start(
                        out=wq_sb[:, pr, k].rearrange("p (u q) -> p u q", u=2),
                        in_=wq_d[
                            2 * pr : 2 * pr + 2, 128 * k : 128 * (k + 1), :
                        ].rearrange("u p q -> p u q"),
                    )
                    nc.sync.dma_start(
                        out=wk_sb[:, pr, k].rearrange("p (u q) -> p u q", u=2),
                        in_=wk_d[
                            2 * pr : 2 * pr + 2, 128 * k : 128 * (k + 1), :
                        ].rearrange("u p q -> p u q"),
                    )
            for k in range(2):
                nc.sync.dma_start(
                    out=wv_sb[:, k],
                    in_=wv_d[:, 128 * k : 128 * (k + 1), :].rearrange("u p m -> p u m"),
                )

            # ---- main loop (final stage software-pipelined by 1 group) ---
            def emit_final(st):
                et_tiles, vw_aug, n, g = st
                # final: unnorm outT per unit, col-packed 4 units/bank
                psum_out = ps_out.tile([16, 4, C], F32, name="psum_out")
                for u4 in range(4):
                    sp, uu = divmod(u4, 2)
                    for j in range(2):
                        nc.tensor.matmul(
                            psum_out[0:MA, u4],
                            vw_aug[:, j, u4],
                            et_tiles[sp][:, uu, j],
                            start=(j == 0),
                            stop=(j == 1),
                        )
                out_sb = outp.tile([16, 4, C], F32, name="out_sb")
                nc.vector.tensor_copy(out_sb[:], psum_out[:])
                nc.gpsimd.dma_start(
                    out=out_d[n, 4 * g : 4 * g + 4].rearrange("u m c -> m u c"),
                    in_=out_sb[0:MA],
                )

            pending = None
            for n in range(NB):
                for g in range(UPC // 4):  # group of 4 units
                    # vW for the 4 units: psum_vw[:, j, u4, m], j = ch chunk
                    psum_vw = ps_vw.tile([128, 2, 4, M], F32, name="psum_vw")
                    for j in range(2):
                        for k in range(2):
                            nc.tensor.matmul(
                                psum_vw[:, j],
                                v_sb[:, n, k, 128 * j : 128 * (j + 1)],
                                wv_sb[:, k, 4 * g : 4 * g + 4],
                                start=(k == 0),
                                stop=(k == 1),
                            )
                    # augmented [p, j, u4, 10]: col 9 = 1.0 (softmax denom row)
                    vw_aug = augp.tile([128, 2, 4, MA], MMDT, name="vw_aug")
                    nc.vector.tensor_copy(vw_aug[:, :, :, 0:M], psum_vw[:])
                    nc.vector.tensor_copy(
                        vw_aug[:, :, :, M:MA], ones_sb.to_broadcast([128, 2, 4, 1])
                    )

                    et_tiles = []
                    for sp in range(2):  # sub-pair of units
                        pr = 2 * g + sp
                        # qWT/kWT 2 units stacked: psum_qk[:,0]=q, [:,1]=k
                        psum_qk = ps_qk.tile([128, 2, C], F32, name="psum_qk")
                        for k in range(2):
                            nc.tensor.matmul(
                                psum_qk[:, 0],
                                wq_sb[:, pr, k],
                                q_sb[:, n, k],
                                start=(k == 0),
                                stop=(k == 1),
                            )
                        for k in range(2):
                            nc.tensor.matmul(
                                psum_qk[:, 1],
                                wk_sb[:, pr, k],
                                v_sb[:, n, k],
                                start=(k == 0),
                                stop=(k == 1),
                            )
                        kq_sb = kqp.tile([128, 2, C], MMDT, name="kq_sb")
                        nc.vector.tensor_copy(kq_sb[:], psum_qk[:])

                        # dotT: [d' chunk j, c] per unit uu; row-group 64*uu
                        psum_dot = ps_dot.tile(
                            [128, 2, 2, C], F32, name="psum_dot"
                        )  # [p, uu, j, c]
                        for uu in range(2):
                            for j in range(2):
                                nc.tensor.matmul(
                                    psum_dot[:, uu, j],
                                    kq_sb[
                                        64 * uu : 64 * uu + 64,
                                        1,
                                        128 * j : 128 * (j + 1),
                                    ],
                                    kq_sb[64 * uu : 64 * uu + 64, 0],
                                    start=True,
                                    stop=True,
                                )
                        et_sb = etp.tile([128, 2, 2, C], ETDT, name="et_sb")
                        nc.scalar.activation(
                            out=et_sb[:],
                            in_=psum_dot[:],
                            func=mybir.ActivationFunctionType.Exp,
                            scale=SCALE,
                        )
                        et_tiles.append(et_sb)

                    if pending is not None:
                        emit_final(pending)
                    pending = (et_tiles, vw_aug, n, g)
            emit_final(pending)

    split_multiwait_drains(nc)
    return nc


_NC_CACHE = None


def _get_nc():
    global _NC_CACHE
    if _NC_CACHE is None:
        _NC_CACHE = build_nc()
    return _NC_CACHE


def make_in_maps(query, value, query_w, key_w, value_w):
    q = np.ascontiguousarray(query.reshape(NB, HW, C), dtype=np.float32)
    v = np.ascontiguousarray(value.reshape(NB, HW, C), dtype=np.float32)
    in_maps = []
    for i in range(N_CORES):
        sl = slice(UPC * i, UPC * (i + 1))
        in_maps.append(
            {
                "query": q,
                "value": v,
                "ones": np.ones((128, 1), dtype=np.float32),
                "query_w": np.ascontiguousarray(query_w[sl], dtype=np.float32),
                "key_w": np.ascontiguousarray(key_w[sl], dtype=np.float32),
                "value_w": np.ascontiguousarray(value_w[sl], dtype=np.float32),
            }
        )
    return in_maps


def gather_output(core_outs):
    """core_outs: list of [NB, UPC, 10, C] -> full [NB, 3, 3, C, 128]."""
    full = np.empty((NB, 3, 3, C, 128), dtype=np.float32)
    for i, o in enumerate(core_outs):
        norm = o[:, :, :M, :] / o[:, :, M : M + 1, :]
        # [n, u, m, c] -> [n, kh, kw, c, u]
        full[:, :, :, :, UPC * i : UPC * (i + 1)] = (
            norm.reshape(NB, UPC, 3, 3, C).transpose(0, 2, 3, 4, 1)
        )
    return full


def kernel(query, value, query_w, key_w, value_w):
    nc = _get_nc()
    in_maps = make_in_maps(query, value, query_w, key_w, value_w)
    res = run_bass_kernel_spmd(nc, in_maps, core_ids=list(range(N_CORES)))
    return gather_output([r["out"] for r in res.results])



# revision 2
# speedup vs baseline: 753.7043x; 753.7043x over previous
"""Trainium2 Bass kernel for nn_CrossAttention_61890478735686.

Math per (batch n, unit u):
    q = query[n] viewed [c=256, hw=256];  raw DRAM layout [hw, c] = q^T
    k = v = value[n] same.
    qW = q @ Wq[u]   [256, 64]
    kW = k @ Wk[u]   [256, 64]
    dot = qW @ kW^T  [256, 256];  attn = softmax(dot/16, axis=-1)
    vW = k @ Wv[u]   [256, 9]
    out = attn @ vW  [256, 9] -> output[n, kh, kw, c, u], m = 3*kh+kw

Kernel dataflow (everything transposed so softmax reduction is the
contraction axis of the final matmul):
    qWT[q, c]  = Wq[u]^T @ q^T      (lhsT = Wq chunks, rhs = raw query)
    kWT[q, c]  = Wk[u]^T @ k^T
    dotT[d, c] = kWT^T-contraction over q (lhsT = kWT cols, rhs = qWT)
    ET = exp(dotT / 16)             (ACT, PSUM -> SBUF)
    unnorm[m, c] = vW_aug^T @ ET    (vW augmented with a ones column ->
                                     row 9 = softmax denominator S[c])
    host: out = unnorm[:9] / unnorm[9]

Sharding: tensor-parallel over units. Core i gets units 16i..16i+16 and
all 16 batches (256 (n,u) pairs per core).
"""

import sys

if "/opt/trn_rl_repo" not in sys.path:
    sys.path.insert(0, "/opt/trn_rl_repo")

import numpy as np

import concourse.bass as bass
import concourse.tile as tile
from concourse import mybir
from concourse.bass_utils import run_bass_kernel_spmd

F32 = mybir.dt.float32
F32R = mybir.dt.float32r

N_CORES = 8
NB = 16          # batches
UPC = 16         # units per core
C = 256          # channels
HW = 256         # h*w (contraction dim of the projections)
QK = 64          # qk_dim
M = 9            # kernel_len
MA = 10          # M + ones column
SCALE = 1.0 / 16.0

USE_F32R = True
MMDT = F32R if USE_F32R else F32
ETDT = MMDT


def split_multiwait_drains(nc):
    """This walrus build cannot codegen instructions carrying >1 sem wait
    (CoreV3GenImpl setupSyncWait: 'Too many sync wait commands').  Hoist
    all but the last wait into single-wait NOPs preceding the instruction
    on the same engine — semantically identical (the sequencer stalls on
    each in turn)."""
    for f in nc.m.functions:
        for bb in f.blocks:
            new_insts = []
            for inst in bb.instructions:
                si = getattr(inst, "sync_info", None)
                if si is not None and len(si.on_wait) > 1:
                    waits = list(si.on_wait)
                    for j, w in enumerate(waits[:-1]):
                        nop = mybir.InstNoOp(
                            name=f"{inst.name}-wsplit{j}",
                            engine=inst.engine,
                            ins=[],
                            outs=[],
                            sync_info=mybir.SyncInfo(on_wait=[w], on_update=[]),
                        )
                        new_insts.append(nop)
                    si.on_wait = [waits[-1]]
                new_insts.append(inst)
            bb.instructions = new_insts


def build_nc():
    nc = bass.Bass()

    q_d = nc.dram_tensor("query", [NB, HW, C], MMDT, kind="ExternalInput")
    v_d = nc.dram_tensor("value", [NB, HW, C], MMDT, kind="ExternalInput")
    wq_d = nc.dram_tensor("query_w", [UPC, HW, QK], MMDT, kind="ExternalInput")
    wk_d = nc.dram_tensor("key_w", [UPC, HW, QK], MMDT, kind="ExternalInput")
    wv_d = nc.dram_tensor("value_w", [UPC, HW, M], MMDT, kind="ExternalInput")
    ones_d = nc.dram_tensor("ones", [128, 1], MMDT, kind="ExternalInput")
    out_d = nc.dram_tensor("out", [NB, UPC, MA, C], F32, kind="ExternalOutput")

    with tile.TileContext(nc) as tc:
        with (
            tc.tile_pool(name="persist", bufs=1) as persist,
            tc.tile_pool(name="kqp", bufs=3) as kqp,
            tc.tile_pool(name="etp", bufs=4) as etp,
            tc.tile_pool(name="augp", bufs=3) as augp,
            tc.tile_pool(name="outp", bufs=3) as outp,
            tc.tile_pool(name="ps_qk", bufs=1, space="PSUM") as ps_qk,
            tc.tile_pool(name="ps_dot", bufs=2, space="PSUM") as ps_dot,
            tc.tile_pool(name="ps_vw", bufs=1, space="PSUM") as ps_vw,
            tc.tile_pool(name="ps_out", bufs=1, space="PSUM") as ps_out,
        ):
            # ---- persistent inputs --------------------------------------
            # q_sb/v_sb: [p=128, n, k, c]; rows (k*128+p) of raw [hw, c]
            q_sb = persist.tile([128, NB, 2, C], MMDT)
            v_sb = persist.tile([128, NB, 2, C], MMDT)
            # wq/wk: [p, pair, k, (u2 qk)]  -> lhsT [128, 128] slices
            wq_sb = persist.tile([128, UPC // 2, 2, 2 * QK], MMDT)
            wk_sb = persist.tile([128, UPC // 2, 2, 2 * QK], MMDT)
            # wv: [p, k, u, m]
            wv_sb = persist.tile([128, 2, UPC, M], MMDT)
            ones_sb = persist.tile([128, 1], MMDT)
            nc.sync.dma_start(out=ones_sb[:], in_=ones_d[:])

            for n in range(NB):
                nc.sync.dma_start(
                    out=q_sb[:, n], in_=q_d[n].rearrange("(k p) c -> p k c", p=128)
                )
                nc.sync.dma_start(
                    out=v_sb[:, n], in_=v_d[n].rearrange("(k p) c -> p k c", p=128)
                )
            for pr in range(UPC // 2):
                for k in range(2):
                    nc.sync.dma_start(
                        out=wq_sb[:, pr, k].rearrange("p (u q) -> p u q", u=2),
                        in_=wq_d[
                            2 * pr : 2 * pr + 2, 128 * k : 128 * (k + 1), :
                        ].rearrange("u p q -> p u q"),
                    )
                    nc.sync.dma_start(
                        out=wk_sb[:, pr, k].rearrange("p (u q) -> p u q", u=2),
                        in_=wk_d[
                            2 * pr : 2 * pr + 2, 128 * k : 128 * (k + 1), :
                        ].rearrange("u p q -> p u q"),
                    )
            for k in range(2):
                nc.sync.dma_start(
                    out=wv_sb[:, k],
                    in_=wv_d[:, 128 * k : 128 * (k + 1), :].rearrange("u p m -> p u m"),
                )

            # ---- main loop (final stage software-pipelined by 1 group) ---
            def emit_final(st):
                et_tiles, vw_aug, n, g = st
                # final: unnorm outT per unit, col-packed 4 units/bank
                psum_out = ps_out.tile([16, 4, C], F32, name="psum_out")
                for u4 in range(4):
                    sp, uu = divmod(u4, 2)
                    for j in range(2):
                        nc.tensor.matmul(
                            psum_out[0:MA, u4],
                            vw_aug[:, j, u4],
                            et_tiles[sp][:, uu, j],
                            start=(j == 0),
                            stop=(j == 1),
                        )
                out_sb = outp.tile([16, 4, C], F32, name="out_sb")
                nc.vector.tensor_copy(out_sb[0:MA], psum_out[0:MA])
                nc.gpsimd.dma_start(
                    out=out_d[n, 4 * g : 4 * g + 4].rearrange("u m c -> m u c"),
                    in_=out_sb[0:MA],
                )

            pending = None
            for n in range(NB):
                for g in range(UPC // 4):  # group of 4 units
                    # vW for the 4 units: psum_vw[:, j, u4, m], j = ch chunk
                    psum_vw = ps_vw.tile([128, 2, 4, M], F32, name="psum_vw")
                    for j in range(2):
                        for k in range(2):
                            nc.tensor.matmul(
                                psum_vw[:, j],
                                v_sb[:, n, k, 128 * j : 128 * (j + 1)],
                                wv_sb[:, k, 4 * g : 4 * g + 4],
                                start=(k == 0),
                                stop=(k == 1),
                            )
                    # augmented [p, j, u4, 10]: col 9 = 1.0 (softmax denom row)
                    vw_aug = augp.tile([128, 2, 4, MA], MMDT, name="vw_aug")
                    nc.vector.tensor_copy(vw_aug[:, :, :, 0:M], psum_vw[:])
                    nc.vector.tensor_copy(
                        vw_aug[:, :, :, M:MA], ones_sb.to_broadcast([128, 2, 4, 1])
                    )

                    et_tiles = []
                    for sp in range(2):  # sub-pair of units
                        pr = 2 * g + sp
                        # qWT/kWT 2 units stacked: psum_qk[:,0]=q, [:,1]=k
                        psum_qk = ps_qk.tile([128, 2, C], F32, name="psum_qk")
                        for k in range(2):
                            nc.tensor.matmul(
                                psum_qk[:, 0],
                                wq_sb[:, pr, k],
                                q_sb[:, n, k],
                                start=(k == 0),
                                stop=(k == 1),
                            )
                        for k in range(2):
                            nc.tensor.matmul(
                                psum_qk[:, 1],
                                wk_sb[:, pr, k],
                                v_sb[:, n, k],
                                start=(k == 0),
                                stop=(k == 1),
                            )
                        kq_sb = kqp.tile([128, 2, C], MMDT, name="kq_sb")
                        nc.vector.tensor_copy(kq_sb[:], psum_qk[:])

                        # dotT: [d' chunk j, c] per unit uu; row-group 64*uu
                        psum_dot = ps_dot.tile(
                            [128, 2, 2, C], F32, name="psum_dot"
                        )  # [p, uu, j, c]
                        for uu in range(2):
                            for j in range(2):
                                nc.tensor.matmul(
                                    psum_dot[:, uu, j],
                                    kq_sb[
                                        64 * uu : 64 * uu + 64,
                                        1,
                                        128 * j : 128 * (j + 1),
                                    ],
                                    kq_sb[64 * uu : 64 * uu + 64, 0],
                                    start=True,
                                    stop=True,
                                )
                        et_sb = etp.tile([128, 2, 2, C], ETDT, name="et_sb")
                        nc.scalar.activation(
                            out=et_sb[:],
                            in_=psum_dot[:],
                            func=mybir.ActivationFunctionType.Exp,
                            scale=SCALE,
                        )
                        et_tiles.append(et_sb)

                    if pending is not None:
                        emit_final(pending)
                    pending = (et_tiles, vw_aug, n, g)
            emit_final(pending)

    split_multiwait_drains(nc)
    return nc


_NC_CACHE = None


def _get_nc():
    global _NC_CACHE
    if _NC_CACHE is None:
        _NC_CACHE = build_nc()
    return _NC_CACHE


def make_in_maps(query, value, query_w, key_w, value_w):
    q = np.ascontiguousarray(query.reshape(NB, HW, C), dtype=np.float32)
    v = np.ascontiguousarray(value.reshape(NB, HW, C), dtype=np.float32)
    in_maps = []
    for i in range(N_CORES):
        sl = slice(UPC * i, UPC * (i + 1))
        in_maps.append(
            {
                "query": q,
                "value": v,
                "ones": np.ones((128, 1), dtype=np.float32),
                "query_w": np.ascontiguousarray(query_w[sl], dtype=np.float32),
                "key_w": np.ascontiguousarray(key_w[sl], dtype=np.float32),
                "value_w": np.ascontiguousarray(value_w[sl], dtype=np.float32),
            }
        )
    return in_maps


def gather_output(core_outs):
    """core_outs: list of [NB, UPC, 10, C] -> full [NB, 3, 3, C, 128]."""
    full = np.empty((NB, 3, 3, C, 128), dtype=np.float32)
    for i, o in enumerate(core_outs):
        norm = o[:, :, :M, :] / o[:, :, M : M + 1, :]
        # [n, u, m, c] -> [n, kh, kw, c, u]
        full[:, :, :, :, UPC * i : UPC * (i + 1)] = (
            norm.reshape(NB, UPC, 3, 3, C).transpose(0, 2, 3, 4, 1)
        )
    return full


def kernel(query, value, query_w, key_w, value_w):
    nc = _get_nc()
    in_maps = make_in_maps(query, value, query_w, key_w, value_w)
    res = run_bass_kernel_spmd(nc, in_maps, core_ids=list(range(N_CORES)))
    return gather_output([r["out"] for r in res.results])



# revision 4
# speedup vs baseline: 1792.0880x; 2.3777x over previous
"""Trainium2 Bass kernel for nn_CrossAttention_61890478735686.

Math per (batch n, unit u):
    q = query[n] viewed [c=256, hw=256];  raw DRAM layout [hw, c] = q^T
    k = v = value[n] same.
    qW = q @ Wq[u]   [256, 64]
    kW = k @ Wk[u]   [256, 64]
    dot = qW @ kW^T  [256, 256];  attn = softmax(dot/16, axis=-1)
    vW = k @ Wv[u]   [256, 9]
    out = attn @ vW  [256, 9] -> output[n, kh, kw, c, u], m = 3*kh+kw

Kernel dataflow (transposed so the softmax axis d is the contraction
of the final matmul):
    qWT[q, c]  = Wq[u]^T @ q^T
    kWT[q, c]  = Wk[u]^T @ k^T
    dotT[d, c] = contraction over q (lhsT = kWT cols, rhs = qWT)
    ET = exp(dotT / 16)
    out2[c, (u, m)] = ET^T @ vw_aug (col 9 of vw_aug = ones -> softmax
                                     denominator row)
    host: out = out2[..., :9] / out2[..., 9]

Key design points (vs the original baseline, measured on HW via
marginal repeat-loop timing):
  - all matmuls in bf16: fp32r does NOT reach 1 cycle/row on real HW
    (measured 2.2x slower end to end); bf16 does. Output rel err ~4e-3,
    well under the 2e-2 gate.
  - final product computed as out2[c',(u,m)] = ET^T @ vw_aug with ET as
    the stationary operand: PSUM output is partition-dense (c'), so the
    PSUM->SBUF copy is a dense [128, 320] DVE copy per batch instead of
    10-partition sparse copies (PSUM matmul writes only support
    partition base 0, so packing units into partitions is impossible).
  - vW for all 16 units in one per-batch matmul group (packed rhs).
  - one staged output DMA per batch on the SP HWDGE queue; no SWDGE
    (Pool) DMAs anywhere - their For_i loop reset (InstIncSwdgeSem)
    does not codegen on this walrus build.
  - input DMAs ordered batch-0-first on SP so the PE starts ~4us in.
  - build_nc(repeat=R) wraps the body in a hardware For_i loop for
    marginal (fixed-overhead-free) HW timing: per-pass 184us vs the
    ~18ms fixed axon/PJRT dispatch overhead per NEFF execution.

Sharding: tensor-parallel over units. Core i gets units 16i..16i+16 and
all 16 batches.
"""

import sys

if "/opt/trn_rl_repo" not in sys.path:
    sys.path.insert(0, "/opt/trn_rl_repo")

import numpy as np

import concourse.bass as bass
import concourse.tile as tile
from concourse import mybir
from concourse.bass_utils import run_bass_kernel_spmd

F32 = mybir.dt.float32
F32R = mybir.dt.float32r

N_CORES = 8
NB = 16          # batches
UPC = 16         # units per core
C = 256          # channels
HW = 256         # h*w (contraction dim of the projections)
QK = 64          # qk_dim
M = 9            # kernel_len
MA = 10          # M + ones column
SCALE = 1.0 / 16.0

BF16 = mybir.dt.bfloat16
MMDT = BF16
ETDT = BF16


def split_multiwait_drains(nc):
    """This walrus build cannot codegen instructions carrying >1 sem wait
    (CoreV3GenImpl setupSyncWait: 'Too many sync wait commands').  Hoist
    all but the last wait into single-wait NOPs preceding the instruction
    on the same engine — semantically identical (the sequencer stalls on
    each in turn)."""
    for f in nc.m.functions:
        for bb in f.blocks:
            new_insts = []
            for inst in bb.instructions:
                si = getattr(inst, "sync_info", None)
                if si is not None and len(si.on_wait) > 1:
                    waits = list(si.on_wait)
                    for j, w in enumerate(waits[:-1]):
                        nop = mybir.InstNoOp(
                            name=f"{inst.name}-wsplit{j}",
                            engine=inst.engine,
                            ins=[],
                            outs=[],
                            sync_info=mybir.SyncInfo(on_wait=[w], on_update=[]),
                        )
                        new_insts.append(nop)
                    si.on_wait = [waits[-1]]
                new_insts.append(inst)
            bb.instructions = new_insts


def build_nc(repeat=1):
    nc = bass.Bass()

    q_d = nc.dram_tensor("query", [NB, HW, C], MMDT, kind="ExternalInput")
    v_d = nc.dram_tensor("value", [NB, HW, C], MMDT, kind="ExternalInput")
    wq_d = nc.dram_tensor("query_w", [UPC, HW, QK], MMDT, kind="ExternalInput")
    wk_d = nc.dram_tensor("key_w", [UPC, HW, QK], MMDT, kind="ExternalInput")
    wv_d = nc.dram_tensor("value_w", [UPC, HW, M], MMDT, kind="ExternalInput")
    ones_d = nc.dram_tensor("ones", [128, 1], MMDT, kind="ExternalInput")
    # out[n, cc, p, u, m]: channel c = 128*cc + p; m=9 = denominator
    out_d = nc.dram_tensor("out", [NB, 2, 128, UPC, MA], F32, kind="ExternalOutput")

    with tile.TileContext(nc) as tc:
        with (
            tc.tile_pool(name="persist", bufs=1) as persist,
            tc.tile_pool(name="kqp", bufs=3) as kqp,
            tc.tile_pool(name="etp", bufs=3) as etp,
            tc.tile_pool(name="stp", bufs=2) as stp,
            tc.tile_pool(name="ps_qk", bufs=2, space="PSUM") as ps_qk,
            tc.tile_pool(name="ps_dot", bufs=2, space="PSUM") as ps_dot,
            tc.tile_pool(name="ps_vw", bufs=1, space="PSUM") as ps_vw,
            tc.tile_pool(name="ps_out", bufs=1, space="PSUM") as ps_out,
        ):
            # ---- persistent tiles ---------------------------------------
            # q_sb/v_sb: [p, n, k, c]; row (k*128+p) of raw [hw, c]
            q_sb = persist.tile([128, NB, 2, C], MMDT)
            v_sb = persist.tile([128, NB, 2, C], MMDT)
            # wq/wk: [p, pair, k, (uu qk)] -> lhsT [128, 128] slices
            wq_sb = persist.tile([128, UPC // 2, 2, 2 * QK], MMDT)
            wk_sb = persist.tile([128, UPC // 2, 2, 2 * QK], MMDT)
            # wv: [p, k, u, m]
            wv_sb = persist.tile([128, 2, UPC, M], MMDT)
            ones_sb = persist.tile([128, 1], MMDT)
            # vw_aug double buffer: [p(d chunk), j, u, MA]; col 9 constant 1.0
            # (the softmax denominator row)
            aug0 = persist.tile([128, 2, UPC, MA], MMDT)
            aug1 = persist.tile([128, 2, UPC, MA], MMDT)
            augs = [aug0, aug1]

            nc.sync.dma_start(out=ones_sb[:], in_=ones_d[:])
            for aug in augs:
                nc.vector.tensor_copy(
                    aug[:, :, :, M:MA],
                    ones_sb.to_broadcast([128, 2, UPC, MA - M]),
                )

            def emit_body():
                # ---- input loads ----------------------------------------
                # No SWDGE (Pool) DMAs anywhere: the For_i repeat wrapper
                # would need an InstIncSwdgeSem loop reset that this walrus
                # build cannot codegen.  SP carries everything, ordered so
                # batch 0 can start ~4us in.
                def load_w(eng, pr):
                    for k in range(2):
                        eng.dma_start(
                            out=wq_sb[:, pr, k].rearrange("p (u q) -> p u q", u=2),
                            in_=wq_d[
                                2 * pr : 2 * pr + 2, 128 * k : 128 * (k + 1), :
                            ].rearrange("u p q -> p u q"),
                        )
                        eng.dma_start(
                            out=wk_sb[:, pr, k].rearrange("p (u q) -> p u q", u=2),
                            in_=wk_d[
                                2 * pr : 2 * pr + 2, 128 * k : 128 * (k + 1), :
                            ].rearrange("u p q -> p u q"),
                        )

                for k in range(2):
                    nc.sync.dma_start(
                        out=wv_sb[:, k],
                        in_=wv_d[:, 128 * k : 128 * (k + 1), :].rearrange(
                            "u p m -> p u m"
                        ),
                    )
                nc.sync.dma_start(
                    out=v_sb[:, 0], in_=v_d[0].rearrange("(k p) c -> p k c", p=128)
                )
                nc.sync.dma_start(
                    out=q_sb[:, 0], in_=q_d[0].rearrange("(k p) c -> p k c", p=128)
                )
                for pr in range(UPC // 2):
                    load_w(nc.sync, pr)
                for n in range(1, NB):
                    nc.sync.dma_start(
                        out=v_sb[:, n],
                        in_=v_d[n].rearrange("(k p) c -> p k c", p=128),
                    )
                    nc.sync.dma_start(
                        out=q_sb[:, n],
                        in_=q_d[n].rearrange("(k p) c -> p k c", p=128),
                    )

                # ---- main loop (final stage pipelined by one step) ------
                def emit_final(st):
                    # final: out2[c', (u, m)] = sum_d ET[d, c'] vw_aug[d, m]
                    # — ET is the stationary operand, so the PSUM output is
                    # partition-dense (c' channels) and the copy is cheap.
                    et_list, aug, n, ublk, psum_o2, stage = st
                    for qq in range(4):
                        u = 4 * ublk + qq
                        sl, uu = divmod(qq, 2)
                        for cc in range(2):
                            for j in range(2):
                                nc.tensor.matmul(
                                    psum_o2[:, cc, u],
                                    et_list[sl][
                                        :, uu, j, 128 * cc : 128 * (cc + 1)
                                    ],
                                    aug[:, j, u],
                                    start=(j == 0),
                                    stop=(j == 1),
                                )
                    if ublk == 3:
                        nc.vector.tensor_copy(stage[:], psum_o2[:])
                        nc.sync.dma_start(
                            out=out_d[n].rearrange("cc p u m -> p cc u m"),
                            in_=stage[:],
                        )

                pending = None
                for n in range(NB):
                    # vW for all 16 units: psum_vw[:, j] = [c chunk j, u, m]
                    psum_vw = ps_vw.tile([128, 2, UPC, M], F32, name="psum_vw")
                    for j in range(2):
                        for k in range(2):
                            nc.tensor.matmul(
                                psum_vw[:, j],
                                v_sb[:, n, k, 128 * j : 128 * (j + 1)],
                                wv_sb[:, k],
                                start=(k == 0),
                                stop=(k == 1),
                            )
                    aug = augs[n % 2]
                    for j in range(2):
                        nc.vector.tensor_copy(aug[:, j, :, 0:M], psum_vw[:, j])

                    psum_o2 = ps_out.tile([128, 2, UPC, MA], F32, name="psum_o2")
                    stage = stp.tile([128, 2, UPC, MA], F32, name="stage")
                    for ublk in range(4):
                        et_list = []
                        for sl in range(2):
                            pr = 2 * ublk + sl
                            psum_qk = ps_qk.tile([128, 2, C], F32, name="psum_qk")
                            for k in range(2):
                                nc.tensor.matmul(
                                    psum_qk[:, 0],
                                    wq_sb[:, pr, k],
                                    q_sb[:, n, k],
                                    start=(k == 0),
                                    stop=(k == 1),
                                )
                            for k in range(2):
                                nc.tensor.matmul(
                                    psum_qk[:, 1],
                                    wk_sb[:, pr, k],
                                    v_sb[:, n, k],
                                    start=(k == 0),
                                    stop=(k == 1),
                                )
                            kq = kqp.tile([128, 2, C], MMDT, name="kq")
                            nc.vector.tensor_copy(kq[:], psum_qk[:])

                            psum_dot = ps_dot.tile(
                                [128, 2, 2, C], F32, name="psum_dot"
                            )
                            for uu in range(2):
                                for j in range(2):
                                    nc.tensor.matmul(
                                        psum_dot[:, uu, j],
                                        kq[
                                            64 * uu : 64 * uu + 64,
                                            1,
                                            128 * j : 128 * (j + 1),
                                        ],
                                        kq[64 * uu : 64 * uu + 64, 0],
                                        start=True,
                                        stop=True,
                                    )
                            et = etp.tile([128, 2, 2, C], ETDT, name="et")
                            nc.scalar.activation(
                                out=et[:],
                                in_=psum_dot[:],
                                func=mybir.ActivationFunctionType.Exp,
                                scale=SCALE,
                            )
                            et_list.append(et)

                        if pending is not None:
                            emit_final(pending)
                        pending = (et_list, aug, n, ublk, psum_o2, stage)
                emit_final(pending)

            if repeat == 1:
                emit_body()
            else:
                with tc.For_i(0, repeat, 1):
                    emit_body()

    split_multiwait_drains(nc)
    return nc


_NC_CACHE = {}


def _get_nc(repeat=1):
    if repeat not in _NC_CACHE:
        _NC_CACHE[repeat] = build_nc(repeat)
    return _NC_CACHE[repeat]


def make_in_maps(query, value, query_w, key_w, value_w):
    import ml_dtypes

    bf = ml_dtypes.bfloat16
    q = np.ascontiguousarray(query.reshape(NB, HW, C)).astype(bf)
    v = np.ascontiguousarray(value.reshape(NB, HW, C)).astype(bf)
    in_maps = []
    for i in range(N_CORES):
        sl = slice(UPC * i, UPC * (i + 1))
        in_maps.append(
            {
                "query": q,
                "value": v,
                "ones": np.ones((128, 1), dtype=bf),
                "query_w": np.ascontiguousarray(query_w[sl]).astype(bf),
                "key_w": np.ascontiguousarray(key_w[sl]).astype(bf),
                "value_w": np.ascontiguousarray(value_w[sl]).astype(bf),
            }
        )
    return in_maps


def gather_output(core_outs):
    """core_outs: list of [NB, 2, 128, UPC, MA] -> full [NB, 3, 3, C, 128].

    out[n, cc, p, u, m] is unnorm for channel c = 128*cc + p
    (m=9 = softmax denominator).
    """
    full = np.empty((NB, 3, 3, C, 128), dtype=np.float32)
    for i, o in enumerate(core_outs):
        norm = o[..., :M] / o[..., M : M + 1]
        # [n, cc, p, u, m(kh kw)] -> [n, kh, kw, c = 128cc+p, u]
        arr = norm.reshape(NB, 2, 128, UPC, 3, 3).transpose(0, 4, 5, 1, 2, 3)
        full[:, :, :, :, UPC * i : UPC * (i + 1)] = arr.reshape(NB, 3, 3, C, UPC)
    return full


def kernel(query, value, query_w, key_w, value_w):
    nc = _get_nc()
    in_maps = make_in_maps(query, value, query_w, key_w, value_w)
    res = run_bass_kernel_spmd(nc, in_maps, core_ids=list(range(N_CORES)))
    return gather_output([r["out"] for r in res.results])
